# revision 10
# baseline (speedup 1.0000x reference)
"""Trainium2 Bass kernel for the NeuralMemory (scatter_memory) problem.

Math (B=1, N=512, D=128, DEPTH=4): per-token meta-gradients of the memory
MLP are rank-1 per layer, so the (n, depth, d, d) momentum/update scans
collapse to a scalar coefficient matrix C[t,s] applied attention-style:

    retrieved_l(t) = y_t @ W_l + sum_s C[t,s]*(-lr_s) * (y_t . x_l(s)) * g_l(s)

The recurrence coefficients decay geometrically, so C is numerically banded:
C[t,s] == 0 (fp32) for t-s >= 64.  Each of the 8 cores handles one 64-query
window [qc, qc+64) with the 128-token key window [qc-64, qc+64) -- fully
data-parallel, no collectives.  Core 0's missing past is zero-padded.

v2 latency restructure vs the original:
  - h1 = (Wk@W0)^T @ seq^T via host-fused wk0: the forward chain starts the
    moment DMA chunk 1 lands instead of waiting for x0+copy.
  - d4 = h4 - v^T computed inside one PSUM accumulation group using a
    host-negated wv; the -v^T start-matmul runs early, off the chain.
  - ACT table load is the scalar queue head (no dummy act needed).
  - silu' chains use the 3-op scalar_tensor_tensor form; sp1 on GpSimd,
    sp2/sp3 on DVE, ordered so sp3 is ready before b3 = wmT3 @ d4.
  - backward transposes are emitted after their backward matmuls (the
    delta chain, not the G transposes, is critical); g0 copy on DVE.
  - scans/izlr/bb run on GpSimd, keeping DVE free for the delta chain.
  - PE is kept busy from ~3.7us (dummy warm matmuls in every real gap) so
    the HAM clock gate lifts 1.2GHz->2.4GHz mid-kernel.
"""

import numpy as np

D = 128
N = 512
DEPTH = 4
NCORES = 8
QW = N // NCORES        # 64 queries per core
SW = 2 * QW             # 128-token key window per core

# column offsets inside the consolidated per-core input tensor (128, ALLW),
# grouped into 4 DMA chunks ordered by first use.
OFF_SEQW = 0                     # chunk 1 [0:512)
OFF_WK0 = 128                    # (Wk @ W_mem[0]) host-fused
OFF_WVN = 256                    # -Wv (negated value projection)
OFF_WM1 = 384
OFF_WM2 = 512                    # chunk 2 [512:1024)
OFF_WQ = 640
OFF_WM3 = 768
OFF_WK = 896
OFF_REPL = 1024                  # chunk 3 [1024:1536): lr*(-2/D) replicated
OFF_REPM = 1152
OFF_REPD = 1280
OFF_ID = 1408
OFF_WMT = 1536                   # chunk 4 [1536:2176): W_mem^T x4, W_mem[0]
OFF_WM0 = 2048
ALLW = 2176

_cache = {}


def _build_program():
    import concourse.mybir as mybir
    from concourse import bacc
    from concourse.tile import TileContext

    f32 = mybir.dt.float32
    fp16 = mybir.dt.float16
    AF = mybir.ActivationFunctionType
    ALU = mybir.AluOpType

    nc = bacc.Bacc("TRN2")

    allin_d = nc.dram_tensor("allin", [D, ALLW], fp16, kind="ExternalInput")
    outT_d = nc.dram_tensor("outT", [D, QW], fp16, kind="ExternalOutput")

    with TileContext(nc) as tc:
        with (
            tc.tile_pool(name="sb", bufs=1) as sb,
            tc.tile_pool(name="ph", bufs=2, space="PSUM") as ph,    # h1,h2,h3
            tc.tile_pool(name="pm", bufs=1, space="PSUM") as pm,    # d4,dec,S*
            tc.tile_pool(name="pp", bufs=2, space="PSUM") as pp,    # x0,q,lrb,amb
            tc.tile_pool(name="pb", bufs=2, space="PSUM") as pb,    # b*,tr*
            tc.tile_pool(name="pr", bufs=1, space="PSUM") as pr,    # warm,r*
        ):
            def sbt(tag, shape=(D, SW), dt=fp16):
                return sb.tile(list(shape), dt, tag=tag, name=tag)

            allin = sbt("allin", (D, ALLW))
            nc.sync.dma_start(out=allin[:, 0:512], in_=allin_d[:, 0:512])
            nc.sync.dma_start(out=allin[:, 512:1024], in_=allin_d[:, 512:1024])
            nc.sync.dma_start(out=allin[:, 1024:1536],
                              in_=allin_d[:, 1024:1536])
            nc.sync.dma_start(out=allin[:, 1536:2176], in_=allin_d[:, 1536:2176])

            seqW = allin[:, OFF_SEQW:OFF_SEQW + SW]
            wk0 = allin[:, OFF_WK0:OFF_WK0 + D]
            wvn = allin[:, OFF_WVN:OFF_WVN + D]
            wm = [allin[:, OFF_WM0:OFF_WM0 + D],
                  allin[:, OFF_WM1:OFF_WM1 + D],
                  allin[:, OFF_WM2:OFF_WM2 + D],
                  allin[:, OFF_WM3:OFF_WM3 + D]]
            wq = allin[:, OFF_WQ:OFF_WQ + D]
            wk = allin[:, OFF_WK:OFF_WK + D]
            rep_lr = allin[:, OFF_REPL:OFF_REPL + D]
            rep_mom = allin[:, OFF_REPM:OFF_REPM + D]
            rep_dec = allin[:, OFF_REPD:OFF_REPD + D]
            idm = allin[:, OFF_ID:OFF_ID + D]
            wmT = [allin[:, OFF_WMT + D * l:OFF_WMT + D * (l + 1)]
                   for l in range(DEPTH)]

            # ---- PE clock warmup: memset a scratch tile on DVE (earliest
            # engine available), then keep the PE array busy with dummy
            # matmuls until the first input chunk lands.
            scrw = sbt("scrw", (D, D))
            nc.vector.memset(scrw, 0.0)
            warm = pr.tile([D, D], f32, tag="r", name="warm")
            for _ in range(7):
                nc.tensor.matmul(warm, scrw, scrw, start=True, stop=True)

            # ---- forward chain head: h1 the instant chunk 1 lands ----
            ps_h1 = ph.tile([D, SW], f32, tag="h", name="h1")
            nc.tensor.matmul(ps_h1, wk0, seqW, start=True, stop=True)
            # d4 group: -v^T lands early, h4 accumulates into it later
            ps_d4 = pm.tile([D, SW], f32, tag="m", name="d4")
            nc.tensor.matmul(ps_d4, wvn, seqW, start=True, stop=False)
            for _ in range(2):
                nc.tensor.matmul(warm, scrw, scrw, start=True, stop=True)
            wsink = sbt("wsink", (D, 8))
            nc.vector.tensor_copy(wsink, warm[:, 0:8])

            x1 = sbt("x1")
            nc.scalar.activation(x1, ps_h1, AF.Silu)
            th1 = sbt("th1")
            nc.scalar.activation(th1, ps_h1, AF.Tanh, scale=0.5)

            # x0 (keys^T) off the critical chain; needed for S0 only
            ps_x0 = pp.tile([D, SW], f32, tag="p", name="x0")
            nc.tensor.matmul(ps_x0, wk, seqW, start=True, stop=True)
            x0 = sbt("x0")
            nc.vector.tensor_copy(x0, ps_x0)

            ps_h2 = ph.tile([D, SW], f32, tag="h", name="h2")
            nc.tensor.matmul(ps_h2, wm[1], x1, start=True, stop=True)

            ps_q = pp.tile([D, QW], f32, tag="p", name="q")
            nc.tensor.matmul(ps_q, wq, seqW[:, QW:SW], start=True, stop=True)
            qT = sbt("qT", (D, QW))
            nc.vector.tensor_copy(qT, ps_q)

            x2 = sbt("x2")
            nc.scalar.activation(x2, ps_h2, AF.Silu)
            th2 = sbt("th2")
            nc.scalar.activation(th2, ps_h2, AF.Tanh, scale=0.5)
            nc.tensor.matmul(warm, scrw, scrw, start=True, stop=True)

            ps_h3 = ph.tile([D, SW], f32, tag="h", name="h3")
            nc.tensor.matmul(ps_h3, wm[2], x2, start=True, stop=True)

            th3 = sbt("th3")
            nc.scalar.activation(th3, ps_h3, AF.Tanh, scale=0.5)
            x3 = sbt("x3")
            nc.scalar.activation(x3, ps_h3, AF.Silu)

            ps_lrb = pp.tile([D, SW], f32, tag="p", name="lrb")
            nc.tensor.matmul(ps_lrb, rep_lr, seqW, start=True, stop=True)

            # h4 accumulates onto -v^T: ps_d4 = h4 - v^T = d4
            nc.tensor.matmul(ps_d4, wm[3], x3, start=False, stop=True)

            ps_amb = pp.tile([D, SW], f32, tag="p", name="amb")
            nc.tensor.matmul(ps_amb, rep_mom, seqW, start=True, stop=True)

            # sp chains (3-op STT form): sp = 0.5*(1+th+x) - 0.5*th*x
            def sp_ops(eng, th, x, pref):
                t = sbt(f"{pref}t")
                eng.scalar_tensor_tensor(t, th, -0.5, x, ALU.mult, ALU.mult)
                w = sbt(f"{pref}w")
                eng.scalar_tensor_tensor(w, th, 1.0, x, ALU.add, ALU.add)
                s = sbt(f"{pref}s")
                eng.scalar_tensor_tensor(s, w, 0.5, t, ALU.mult, ALU.add)
                return s

            # sp1/sp2 on GpSimd via the 4-op form (no STT support there):
            # s = 0.5+0.5*th; sp = s + x - x*s
            def sp_gp(th, x, pref):
                s = sbt(f"{pref}s")
                nc.gpsimd.tensor_scalar(s, th, 0.5, 0.5, ALU.mult, ALU.add)
                xs = sbt(f"{pref}xs")
                nc.gpsimd.tensor_mul(xs, x, s)
                u = sbt(f"{pref}u")
                nc.gpsimd.tensor_sub(u, x, xs)
                sp = sbt(f"{pref}sp")
                nc.gpsimd.tensor_add(sp, s, u)
                return sp

            sp1 = sp_gp(th1, x1, "s1")
            sp2 = sp_gp(th2, x2, "s2")
            # sp3 on DVE -- it gates d3
            sp3 = sp_ops(nc.vector, th3, x3, "s3")

            # d4 -> SBUF on Scalar (GpSimd cannot read PSUM; DVE is busy
            # with the sp chains)
            d4 = sbt("d4")
            nc.scalar.copy(d4, ps_d4)

            # ---- backward deltas; transposes follow their matmuls ----
            G = [None] * DEPTH
            Dl = [None] * (DEPTH + 1)
            Dl[4] = d4
            for l in range(DEPTH - 1, -1, -1):
                if l > 0:
                    ps_b = pb.tile([D, SW], f32, tag="bt", name=f"b{l}")
                    nc.tensor.matmul(ps_b, wmT[l], Dl[l + 1], start=True,
                                     stop=True)
                    SPl = {3: sp3, 2: sp2, 1: sp1}[l]
                    dl = sbt(f"d{l}")
                    nc.vector.tensor_mul(dl, ps_b, SPl)
                    Dl[l] = dl
                ps_t = pb.tile([D, D], fp16, tag="bt", name=f"t{l}")
                nc.tensor.transpose(ps_t, Dl[l + 1], idm)
                gl = sbt(f"g{l}")
                if l == 0:
                    nc.vector.tensor_copy(gl, ps_t)
                else:
                    nc.scalar.copy(gl, ps_t)
                G[l] = gl

            # ---- decay path + scans (GpSimd; DVE owns the delta chain) ----
            ps_dec = pm.tile([D, SW], f32, tag="m", name="dec")
            nc.tensor.matmul(ps_dec, rep_dec, seqW, start=True, stop=True)
            th_dec = sbt("th_dec")
            nc.scalar.activation(th_dec, ps_dec, AF.Tanh, scale=0.5)
            bb = sbt("bb")
            nc.gpsimd.tensor_scalar(bb, th_dec, -0.5, 0.5, ALU.mult, ALU.add)
            # GpSimd cannot read PSUM: stage lrb/amb through SBUF on Scalar
            lrbs = sbt("lrbs")
            nc.scalar.copy(lrbs, ps_lrb)
            ambs = sbt("ambs")
            nc.scalar.copy(ambs, ps_amb)
            izlr = sbt("izlr")
            nc.gpsimd.tensor_mul(izlr, idm, lrbs)
            # scans only exist on DVE
            AT = sbt("AT")
            nc.vector.tensor_tensor_scan(AT, ambs, izlr, 0.0,
                                         ALU.mult, ALU.add)
            CT = sbt("CT")
            nc.vector.tensor_tensor_scan(CT, bb, AT, 0.0, ALU.mult, ALU.add)

            # ---- retrieval over this core's 64-query window ----
            Y = qT
            CTq = CT[:, QW:SW]
            for l in range(DEPTH):
                ps_s = pm.tile([D, QW], f32, tag="m", name=f"S{l}")
                nc.tensor.matmul(ps_s, [x0, x1, x2, x3][l], Y, start=True,
                                 stop=True)
                cst = sbt(f"cst{l}", (D, QW))
                nc.vector.tensor_mul(cst, ps_s, CTq)
                ps_o = pr.tile([D, QW], f32, tag="r", name=f"r{l}")
                nc.tensor.matmul(ps_o, wm[l], Y, start=True, stop=False)
                nc.tensor.matmul(ps_o, G[l], cst, start=False, stop=True)
                if l < DEPTH - 1:
                    ynext = sbt(f"y{l + 1}", (D, QW))
                    nc.scalar.activation(ynext, ps_o, AF.Silu)
                    Y = ynext
                else:
                    outT = sbt("outT", (D, QW), dt=fp16)
                    h = QW // 2
                    nc.vector.tensor_copy(outT[:, 0:h], ps_o[:, 0:h])
                    nc.sync.dma_start(out=outT_d[:, 0:h], in_=outT[:, 0:h])
                    nc.vector.tensor_copy(outT[:, h:QW], ps_o[:, h:QW])
                    nc.scalar.dma_start(out=outT_d[:, h:QW],
                                        in_=outT[:, h:QW])

    return nc


def get_program():
    if "nc" not in _cache:
        nc = _build_program()
        nc.finalize()
        _cache["nc"] = nc
    return _cache["nc"]


def make_in_maps(seq, W_mem, W_q, W_kv, W_mom, W_step, W_decay):
    seq = np.asarray(seq, dtype=np.float32)
    W_mem = np.asarray(W_mem, dtype=np.float32)
    W_kv = np.asarray(W_kv, dtype=np.float32)
    seqT = seq.reshape(N, D).T  # (d, n)

    base = np.zeros((D, ALLW), dtype=np.float16)
    base[:, OFF_WK0:OFF_WK0 + D] = W_kv[:, :D] @ W_mem[0]
    base[:, OFF_WVN:OFF_WVN + D] = -W_kv[:, D:]
    base[:, OFF_WK:OFF_WK + D] = W_kv[:, :D]
    base[:, OFF_WQ:OFF_WQ + D] = np.asarray(W_q, dtype=np.float32)
    for l in range(DEPTH):
        off = [OFF_WM0, OFF_WM1, OFF_WM2, OFF_WM3][l]
        base[:, off:off + D] = W_mem[l]
        base[:, OFF_WMT + D * l:OFF_WMT + D * (l + 1)] = W_mem[l].T
    lr_col = np.asarray(W_step, dtype=np.float32)[:, 0] * (-2.0 / D)
    base[:, OFF_REPL:OFF_REPL + D] = np.repeat(lr_col[:, None], D, axis=1)
    base[:, OFF_REPM:OFF_REPM + D] = np.repeat(
        np.asarray(W_mom, dtype=np.float32)[:, :1], D, axis=1)
    base[:, OFF_REPD:OFF_REPD + D] = np.repeat(
        np.asarray(W_decay, dtype=np.float32)[:, :1], D, axis=1)
    base[:, OFF_ID:OFF_ID + D] = np.eye(D, dtype=np.float32)

    in_maps = []
    for c in range(NCORES):
        allin = base.copy()
        qc = c * QW
        lo = qc - QW
        win = np.zeros((D, SW), dtype=np.float16)
        src_lo = max(lo, 0)
        win[:, src_lo - lo:] = seqT[:, src_lo:qc + QW].astype(np.float16)
        allin[:, OFF_SEQW:OFF_SEQW + SW] = win
        in_maps.append({"allin": allin})
    return in_maps


def assemble(results):
    out = np.empty((N, D), dtype=np.float32)
    for c in range(NCORES):
        out[c * QW:(c + 1) * QW, :] = results[c]["outT"].T.astype(np.float32)
    return out.reshape(1, N, D)


def kernel(**inputs) -> np.ndarray:
    from concourse.bass_utils import run_bass_kernel_spmd

    nc = get_program()
    in_maps = make_in_maps(**inputs)
    res = run_bass_kernel_spmd(nc, in_maps, list(range(NCORES)))
    return assemble(res.results)
